# revision 33
# baseline (speedup 1.0000x reference)
"""Chamfer distance kernel for Trainium2 (8 NeuronCores, data-parallel over batch).

Input : x, y float32 [16, 4096, 3]
Output: scalar float32 = mean_b [ mean_n min_m ||x_bn - y_bm||^2
                                + mean_m min_n ||x_bn - y_bm||^2 ]

This environment charges a large, roughly flat cost per *instruction*
plus ~1.3 ns per element (engines do not overlap), so the kernel
minimizes instruction count by computing squared distances directly on
the Vector engine with giant multi-dim access patterns (up to 128 x 64K
elements per op, stride-0 broadcasts), instead of PE matmuls (capped at
512 columns per instruction, which would need 512+ instructions/core).

Layout per core (2 batches). x points live along partitions (32 blocks
of 128 rows); y is replicated across partitions via a partition-broadcast
DMA. The pair space is processed in 4 m-chunks of 1024 columns covering
all 32 blocks at once; c[p, g*2056 + m*2 + k] holds two k-interleaved
diff slots per (g, m) (the pad keeps merged APs under the 16-bit ISA
num_elem limit). Per (batch, chunk) - 6 Vector ops total:
  c[.,k01]   = x_k - y_k      (k=0,1)       1 TT sub, 65536 elements
  c[.,k0]    = c0^2 + c1^2                  1 custom DVE op (SQSQ)
  c[.,k1]    = x_2 - y_2                    1 TT sub
  u = c[.,k0] += c[.,k1]^2                  1 custom DVE op (ADDSQ)
  rowacc     = min_m u   [128, 32]          1 segmented reduce
  colout     = min_g u   [128, 1024]        1 strided reduce
The host does the final 128-partition min + mean (tiny numpy):
cross-partition DVE inputs are illegal on this target and a DMA out is
one instruction versus a 14-op on-device tree.
"""
import sys

sys.path.insert(0, "/opt/trn_rl_repo")

import numpy as np

import concourse.bacc as bacc
import concourse.tile as tile
from concourse import mybir
from concourse.alu_op_type import AluOpType
from concourse.bass_utils import run_bass_kernel_spmd

# --- custom DVE ops (registered at import time) ---------------------------
import concourse.dve_ops as dve_ops
from concourse.dve_ops import DveOp
from concourse.dve_spec import Spec, Src0, Src1, sq, lower, _has_src1


def _register_dve_op(name, spec):
    if name in dve_ops._SUB_OPCODE_FOR_NAME:
        for o in dve_ops.OPS:
            if o.name == name:
                return o
    row = dve_ops._CUSTOM_DVE_ROW_BASE + len(dve_ops.OPS)
    assert row < 0x20
    dve_ops._SUB_OPCODE_FOR_NAME[name] = row
    from concourse.dve_uop import DveOpSpec

    shas = {}
    for ver in ("v3", "v4"):
        try:
            uops = lower(spec, ver=ver)
            s = DveOpSpec(name=name, opcode=row, uops=uops, rd1_en=_has_src1(spec))
            shas[ver] = s.sha(ver)
        except Exception:
            pass
    op = DveOp(name, spec, subdim=False, uops_sha=shas)
    dve_ops.OPS.append(op)
    dve_ops.CUSTOM_DVE_SPECS[name] = spec
    return op


SQSQ = _register_dve_op(
    "SQSQ_ANT",
    Spec(
        body=sq(Src0) + sq(Src1),
        reference=lambda in0, in1, s0, s1, imm2: (
            in0.astype(np.float32) ** 2 + in1.astype(np.float32) ** 2
        ),
    ),
)
ADDSQ = _register_dve_op(
    "ADDSQ_ANT",
    Spec(
        body=Src0 + sq(Src1),
        reference=lambda in0, in1, s0, s1, imm2: (
            in0.astype(np.float32) + in1.astype(np.float32) ** 2
        ),
    ),
)
# ---------------------------------------------------------------------------

F32 = mybir.dt.float32
F16 = mybir.dt.float16
X = mybir.AxisListType.X
A = AluOpType

B, N, KC = 16, 4096, 3
NCORES = 8
BPC = B // NCORES            # batches per core
NBLK = N // 128              # 32 x-blocks per batch
MC = 4                       # m-chunks per batch
CM = N // MC                 # 1024 columns per chunk
GS = 2 * CM + 8              # per-block stride in c: two contiguous k-subplanes
                             # of CM plus pad (stops full AP merge, which would
                             # overflow the 16-bit num_elem field)
CW = NBLK * GS               # c tile width


def _build_nc(repeat: int = 1):
    nc = bacc.Bacc("TRN2", target_bir_lowering=False, debug=False, num_devices=NCORES)
    xp_d = nc.dram_tensor("xp", [128, BPC * NBLK * KC], F16, kind="ExternalInput").ap()
    yp_d = nc.dram_tensor("yp", [1, BPC * N * KC], F16, kind="ExternalInput").ap()
    col_d = nc.dram_tensor("col", [128, BPC * N], F16, kind="ExternalOutput").ap()
    row_d = nc.dram_tensor("row", [128, BPC * MC * NBLK], F32, kind="ExternalOutput").ap()

    with tile.TileContext(nc) as tc:
        import contextlib
        with contextlib.ExitStack() as ctx:
            const = ctx.enter_context(tc.tile_pool(name="const", bufs=1))

            xp_t = const.tile([128, BPC * NBLK * KC], F16, name="xp_t")
            nc.scalar.dma_start(xp_t[:], xp_d[:])
            yp_t = const.tile([128, BPC * N * KC], F16, name="yp_t")
            nc.scalar.dma_start(
                yp_t[:], yp_d[0:1, :].partition_broadcast(128).squeeze(1))
            c_t = const.tile([128, CW], F16, name="c_t")
            colout = const.tile([128, BPC * N], F16, name="colout")
            rowacc = const.tile([128, BPC * MC * NBLK], F32, name="rowacc")

            # c layout [p, g, k, m] (per-g contiguous k-subplanes, padded apart)
            cgq = c_t[:].rearrange("p (g q) -> p g q", g=NBLK)[:, :, 0:2 * CM]
            cgkm = cgq.rearrange("p g (k m) -> p g k m", k=2)
            c4 = cgq.rearrange("p g (k m) -> p k g m", k=2)   # sub iterates k,g,m
            u_v = cgkm[:, :, 0, :]                    # [128, NBLK, CM] k=0
            v_v = cgkm[:, :, 1, :]                    # [128, NBLK, CM] k=1
            cB = c4[:, 1:2, :, :]
            cmg = (
                c_t[:].rearrange("p (g q) -> p q g", g=NBLK)[:, 0:CM, :]
            )                                          # [128, CM, NBLK] k=0 slots

            for _rep in range(repeat):
                for b in range(BPC):
                    xkg = xp_t[:, b * NBLK * KC:(b + 1) * NBLK * KC] \
                        .rearrange("p (k g) -> p k g", k=KC)
                    x_apA = xkg[:, 0:2, :].unsqueeze(3).broadcast_to([128, 2, NBLK, CM])
                    x_apB = xkg[:, 2:3, :].unsqueeze(3).broadcast_to([128, 1, NBLK, CM])
                    ykm = yp_t[:, b * N * KC:(b + 1) * N * KC] \
                        .rearrange("p (k m) -> p k m", k=KC)
                    for mc in range(MC):
                        ymc = ykm[:, :, mc * CM:(mc + 1) * CM]
                        y_apA = ymc[:, 0:2, :].unsqueeze(2).broadcast_to([128, 2, NBLK, CM])
                        y_apB = ymc[:, 2:3, :].unsqueeze(2).broadcast_to([128, 1, NBLK, CM])
                        # k subplanes 0,1 = (x0-y0), (x1-y1)
                        nc.vector.tensor_tensor(c4, x_apA, y_apA, op=A.subtract)
                        # k0 = d0^2 + d1^2
                        nc.vector._custom_dve(SQSQ, out=u_v, in0=u_v, in1=v_v)
                        # k1 = (x2-y2)
                        nc.vector.tensor_tensor(cB, x_apB, y_apB, op=A.subtract)
                        # k0 += k1^2  -> u
                        nc.vector._custom_dve(ADDSQ, out=u_v, in0=u_v, in1=v_v)
                        # row direction: min over this chunk's m per (p, g)
                        nc.vector.tensor_reduce(
                            rowacc[:, (b * MC + mc) * NBLK:(b * MC + mc + 1) * NBLK],
                            u_v, axis=X, op=A.min)
                        # col direction: min over all 32 blocks per column
                        nc.vector.tensor_reduce(
                            colout[:, b * N + mc * CM: b * N + (mc + 1) * CM],
                            cmg, axis=X, op=A.min)

            nc.scalar.dma_start(col_d[:], colout[:])
            nc.scalar.dma_start(row_d[:], rowacc[:])
    nc.compile()
    return nc


def _build_operands(x, y):
    """x,y [B,N,3] f32 -> per-core input maps (f16 packed layouts)."""
    x = np.asarray(x, np.float32).astype(np.float16)
    y = np.asarray(y, np.float32).astype(np.float16)
    in_maps = []
    for core in range(NCORES):
        xp = np.empty((128, BPC * NBLK * KC), np.float16)
        yp = np.empty((1, BPC * N * KC), np.float16)
        for j in range(BPC):
            bg = core * BPC + j
            # xp[p, j*96 + k*NBLK + r] = x[bg, r*128 + p, k]  (coordinate-planar)
            xb = x[bg].reshape(NBLK, 128, KC).transpose(1, 2, 0).reshape(128, KC * NBLK)
            xp[:, j * NBLK * KC:(j + 1) * NBLK * KC] = xb
            # yp[j*12288 + k*N + m] = y[bg, m, k]  (coordinate-planar)
            yp[0, j * N * KC:(j + 1) * N * KC] = y[bg].T.reshape(-1)
        in_maps.append({"xp": xp, "yp": yp})
    return in_maps


_NC_CACHE = {}


def _get_nc(repeat: int = 1):
    if repeat not in _NC_CACHE:
        _NC_CACHE[repeat] = _build_nc(repeat)
    return _NC_CACHE[repeat]


def _finalize(results):
    total = 0.0
    for core in range(NCORES):
        row = np.asarray(results[core]["row"], np.float32)   # [128, BPC*MC*NBLK]
        col = np.asarray(results[core]["col"], np.float32)   # [128, BPC*N]
        row = row.reshape(128, BPC, MC, NBLK).min(axis=2)    # min across m-chunks
        for j in range(BPC):
            rsum = row[:, j, :].sum(dtype=np.float64)
            csum = col[:, j * N:(j + 1) * N].min(axis=0).sum(dtype=np.float64)
            total += (rsum + csum) / N
    return np.float32(total / B)


def kernel(x, y):
    x = np.asarray(x, dtype=np.float32)
    y = np.asarray(y, dtype=np.float32)
    assert x.shape == (B, N, KC) and y.shape == (B, N, KC)
    in_maps = _build_operands(x, y)
    nc = _get_nc(1)
    res = run_bass_kernel_spmd(nc, in_maps, core_ids=list(range(NCORES)))
    return _finalize(res.results)
